# revision 24
# baseline (speedup 1.0000x reference)
"""Multi-Head Latent Attention (MLA) Trainium2 kernel, 8-core head-sharded.

v2.2: latent down-projections folded into per-head up-projections on the
host (W_xQ = (W_UQ_h W_DQ)^T etc.) so each core runs only its 2 heads'
fused projections straight from x. All matmul operands bf16 (same PE rate
as f32r, half the SBUF/DMA), f32 PSUM accumulation. Weights and x are
packed host-side into single contiguous mega-tiles so each loads with one
DMA descriptor. Engine split: Scalar(ACT)=exp only, Vector=clips/ropes/
evacs, GpSimd=softmax-sum accumulation + second evac lane. Softmax
denominators accumulate on GpSimd (no per-chunk ones-matmul), collapsed
once per block by one matmul; reciprocal via the fast DVE approximation.

PE order: P0 A0 P1 A1 O0 P2 A2 O1 P3 A3 O2 O3 — each out-projection is
deferred one block so every softmax tail (including the last block's)
hides under independent PE work.
"""
import sys

sys.path.insert(0, "/opt/trn_rl_repo")

import numpy as np
import ml_dtypes

import concourse.bass as bass
import concourse.tile as tile
from concourse import bacc, mybir
from concourse.bass_utils import run_bass_kernel_spmd

F32 = mybir.dt.float32
F32R = mybir.dt.float32r
BF16 = mybir.dt.bfloat16
AF = mybir.ActivationFunctionType
OP = mybir.AluOpType

N_CORES = 8
S = 2048          # sequence length
DM = 2048         # d_model
H = 16            # total heads
HC = H // N_CORES  # heads per core (2)
DH = 128          # head dim (content)
DHR = 64          # head dim (rope)
QB = 512          # query block
NQB = S // QB     # 4
NMC = DM // 128   # 16 model chunks
NKC = S // 128    # 16 key chunks
THETA = 10000.0

SCALE = float(1.0 / np.sqrt(np.float32(DH + DHR)))
E_HI = float(np.exp(np.float64(80.0) * SCALE))
E_LO = float(np.exp(np.float64(-80.0) * SCALE))

# Set by test.py to profile; harness path leaves these untouched.
TRACE = False
TRACE_KWARGS = {}
LAST_EXEC_TIME_NS = None
LAST_RESULTS = None

_CACHE = {}


def _build():
    nc = bacc.Bacc("TRN2", target_bir_lowering=False, debug=False,
                   enable_asserts=True, num_devices=N_CORES)

    def din(name, shape, dt=BF16):
        return nc.dram_tensor(name, shape, dt, kind="ExternalInput").ap()

    d = {
        "xq": din("xq", [128, NQB * NMC * QB]),
        "wq": din("wq", [128, NMC * HC * DH]),
        "wk": din("wk", [128, NMC * HC * DH]),
        "wv": din("wv", [128, NMC * HC * DH]),
        "wr": din("wr", [128, NMC * (HC + 1) * DHR]),
        "woT": din("woT", [HC * DH, DM]),
        "ones128": din("ones128", [128, 1], F32R),
        "ones1": din("ones1", [1, 128], F32R),
        "masktri": din("masktri", [128, 128], F32),
        "cs1": din("cs1", [128, S], F32),
        "cs2": din("cs2", [128, S], F32),
        "outT": nc.dram_tensor("outT", [DM, S], F32,
                               kind="ExternalOutput").ap(),
    }
    with tile.TileContext(nc) as tc:
        import contextlib
        with contextlib.ExitStack() as ctx:
            _kernel_body(ctx, tc, nc, d)
    nc.compile()
    return nc


def _kernel_body(ctx, tc, nc, d):
    wts = ctx.enter_context(tc.tile_pool(name="wts", bufs=1))
    kvp = ctx.enter_context(tc.tile_pool(name="kvp", bufs=1))
    xtp = ctx.enter_context(tc.tile_pool(name="xtp", bufs=2))
    prj = ctx.enter_context(tc.tile_pool(name="prj", bufs=2))
    smp = ctx.enter_context(tc.tile_pool(name="smp", bufs=1))
    o5p = ctx.enter_context(tc.tile_pool(name="o5p", bufs=4))
    ps_p = ctx.enter_context(tc.tile_pool(name="ps_p", bufs=3, space="PSUM"))
    ps_s = ctx.enter_context(tc.tile_pool(name="ps_s", bufs=2, space="PSUM"))
    ps_at = ctx.enter_context(tc.tile_pool(name="ps_at", bufs=2, space="PSUM"))

    # ---- persistent loads: one DMA per mega-tile, spread across queues ----
    wq_t = wts.tile([128, NMC * HC * DH], BF16, name="wq")
    wr_t = wts.tile([128, NMC * (HC + 1) * DHR], BF16, name="wr")
    nc.gpsimd.dma_start(wq_t[:], d["wq"][:, :])
    nc.gpsimd.dma_start(wr_t[:], d["wr"][:, :])
    wk_t = wts.tile([128, NMC * HC * DH], BF16, name="wk")
    wv_t = wts.tile([128, NMC * HC * DH], BF16, name="wv")
    wo_t = [wts.tile([128, DM], BF16, name=f"wo{h}") for h in range(HC)]
    nc.scalar.dma_start(wk_t[:], d["wk"][:, :])
    nc.scalar.dma_start(wv_t[:], d["wv"][:, :])
    for h in range(HC):
        nc.scalar.dma_start(wo_t[h][:], d["woT"][h * 128:(h + 1) * 128, :])
    o128_t = wts.tile([128, 1], F32R, name="o128")
    o1_t = wts.tile([1, 128], F32R, name="o1")
    mask_t = wts.tile([128, 128], F32, name="masktri")
    nc.sync.dma_start(o128_t[:], d["ones128"][:, :])
    nc.sync.dma_start(o1_t[:], d["ones1"][:, :])
    nc.sync.dma_start(mask_t[:], d["masktri"][:, :])

    # ---- persistent per-sequence state (bf16) ----
    kct = [kvp.tile([128, S], BF16, name=f"kct{h}") for h in range(HC)]
    krt2 = kvp.tile([128, S], BF16, name="krt2")  # rope-k duplicated 2x64
    vt = [kvp.tile([128, HC * DH], BF16, name=f"vt{k}") for k in range(NKC)]

    eng = [0]

    def copy_out(dst, src):
        (nc.vector.tensor_copy if eng[0] % 2 == 0
         else nc.scalar.copy)(dst, src)
        eng[0] += 1

    def rope_packed(raw_ps, out_ap, cs1s, cs2s, tag):
        """raw_ps: PSUM [128, QB] pre-rope (two 64-blocks); out: bf16."""
        raw = smp.tile([128, QB], F32, tag="rope_raw", bufs=2,
                       name=f"rr_{tag}")
        nc.vector.tensor_copy(raw[:], raw_ps[:])
        rsw = smp.tile([128, QB], F32, tag="rope_swp", bufs=2,
                       name=f"rs_{tag}")
        for b in range(0, 128, 64):
            nc.sync.dma_start(rsw[b:b + 32, :], raw[b + 32:b + 64, :])
            nc.sync.dma_start(rsw[b + 32:b + 64, :], raw[b:b + 32, :])
        nc.vector.tensor_tensor(raw[:], raw[:], cs1s[:], op=OP.mult)
        nc.vector.tensor_tensor(rsw[:], rsw[:], cs2s[:], op=OP.mult)
        nc.vector.tensor_tensor(out_ap, raw[:], rsw[:], op=OP.add)

    def proj(qb, mid_hook=None):
        """Fused projections for block qb: q/k/v/qr/kr straight from x."""
        qsl = slice(qb * QB, (qb + 1) * QB)
        xq = xtp.tile([128, NMC * QB], BF16, tag="xq", name=f"xq_{qb}")
        # 4 DMAs of 4 chunks each: the first pass starts after 1/4 arrives
        qblk = NMC * QB * qb
        for g in range(4):
            nc.sync.dma_start(
                xq[:, g * 4 * QB:(g + 1) * 4 * QB],
                d["xq"][:, qblk + g * 4 * QB:qblk + (g + 1) * 4 * QB])
        cs1s = smp.tile([128, QB], F32, tag="cs1s", bufs=2, name=f"cs1s{qb}")
        cs2s = smp.tile([128, QB], F32, tag="cs2s", bufs=2, name=f"cs2s{qb}")
        nc.sync.dma_start(cs1s[:], d["cs1"][:, qsl])
        nc.sync.dma_start(cs2s[:], d["cs2"][:, qsl])

        def xm(m):
            return xq[:, m * QB:(m + 1) * QB]

        qct = [prj.tile([128, QB], BF16, tag=f"qct{h}", name=f"qct{h}_{qb}")
               for h in range(HC)]
        qrt = prj.tile([128, QB], BF16, tag="qrt", name=f"qrt_{qb}")
        # q passes then qr (rope DVE work spreads under the k passes)
        for h in range(HC):
            pq = ps_p.tile([128, QB], F32, tag="pp", name=f"pq{h}_{qb}")
            for m in range(NMC):
                nc.tensor.matmul(pq[:],
                                 wq_t[:, m * 256 + h * DH:
                                      m * 256 + (h + 1) * DH],
                                 xm(m), start=(m == 0), stop=(m == NMC - 1))
            copy_out(qct[h][:], pq[:])
        pqr = ps_p.tile([128, QB], F32, tag="pp", name=f"pqr_{qb}")
        for m in range(NMC):
            nc.tensor.matmul(pqr[:], wr_t[:, m * 192:m * 192 + 128],
                             xm(m), start=(m == 0), stop=(m == NMC - 1))
        rope_packed(pqr, qrt[:], cs1s, cs2s, f"qr{qb}")
        # k passes then kr (kr rope spreads under the v pass)
        for h in range(HC):
            pk = ps_p.tile([128, QB], F32, tag="pp", name=f"pk{h}_{qb}")
            for m in range(NMC):
                nc.tensor.matmul(pk[:],
                                 wk_t[:, m * 256 + h * DH:
                                      m * 256 + (h + 1) * DH],
                                 xm(m), start=(m == 0), stop=(m == NMC - 1))
            copy_out(kct[h][:, qsl], pk[:])
        pkrt = ps_p.tile([128, QB], F32, tag="pp", name=f"pkr_{qb}")
        pkr = pkrt[0:64, :]
        for m in range(NMC):
            nc.tensor.matmul(pkr, wr_t[:, m * 192 + 128:m * 192 + 192],
                             xm(m), start=(m == 0), stop=(m == NMC - 1))
        if mid_hook is not None:
            mid_hook()
        krd = smp.tile([128, QB], F32, tag="krd", bufs=2, name=f"krd_{qb}")
        nc.vector.tensor_copy(krd[0:64, :], pkr)
        nc.scalar.copy(krd[64:128, :], pkr)
        pkr2 = smp.tile([128, QB], F32, tag="krd2", bufs=2, name=f"krd2_{qb}")
        for b in range(0, 128, 64):
            nc.sync.dma_start(pkr2[b:b + 32, :], krd[b + 32:b + 64, :])
            nc.sync.dma_start(pkr2[b + 32:b + 64, :], krd[b:b + 32, :])
        nc.vector.tensor_tensor(krd[:], krd[:], cs1s[:], op=OP.mult)
        nc.vector.tensor_tensor(pkr2[:], pkr2[:], cs2s[:], op=OP.mult)
        nc.vector.tensor_tensor(krt2[:, qsl], krd[:], pkr2[:], op=OP.add)
        # v pass (natural [keys, HC*DH] layout)
        for sc in range(QB // 128):
            pv = ps_p.tile([128, HC * DH], F32, tag="pp", name=f"pv{sc}_{qb}")
            for m in range(NMC):
                nc.tensor.matmul(pv[:],
                                 xq[:, m * QB + sc * 128:
                                    m * QB + (sc + 1) * 128],
                                 wv_t[:, m * 256:(m + 1) * 256],
                                 start=(m == 0), stop=(m == NMC - 1))
            copy_out(vt[qb * (QB // 128) + sc][:], pv[:])
        return qct, qrt

    def attn(qb, qct, qrt):
        """Both heads interleaved per key chunk; GpSimd-accumulated sums."""
        nkc = (QB // 128) * (qb + 1)
        pat = [ps_at.tile([128, QB], F32, tag="at", name=f"pat{h}_{qb}")
               for h in range(HC)]
        sumacc = [smp.tile([128, QB], F32R, tag=f"sum{h}", bufs=2,
                           name=f"sum{h}_{qb}") for h in range(HC)]
        pend = []  # (h, kc, off, pt) awaiting PV

        def flush(last):
            h, kc, off, pt = pend.pop(0)
            nc.tensor.matmul(pat[h][:, off:],
                             vt[kc][:, h * DH:(h + 1) * DH],
                             pt[:, off:], start=(kc == 0), stop=last,
                             skip_group_check=True)

        for kc in range(nkc):
            off = 128 * (kc - (QB // 128) * qb) if kc >= (QB // 128) * qb else 0
            w = QB - off
            ksl = slice(kc * 128, (kc + 1) * 128)
            for h in range(HC):
                pss = ps_s.tile([128, QB], F32, tag="ss", name=f"s{h}_{qb}_{kc}")
                nc.tensor.matmul(pss[:, off:], kct[h][:, ksl],
                                 qct[h][:, off:], start=True, stop=False,
                                 skip_group_check=True)
                nc.tensor.matmul(pss[:, off:],
                                 krt2[h * DHR:(h + 1) * DHR, ksl],
                                 qrt[h * DHR:(h + 1) * DHR, off:],
                                 start=False, stop=True,
                                 skip_group_check=True)
                if len(pend) >= 2:
                    flush(False)
                et = smp.tile([128, QB], F32, tag="et", bufs=3,
                              name=f"et{h}_{qb}_{kc}")
                nc.scalar.activation(et[:, off:], pss[:, off:], AF.Exp,
                                     scale=SCALE)
                pt = smp.tile([128, QB], BF16, tag="pt", bufs=4,
                              name=f"pt{h}_{qb}_{kc}")
                if kc >= (QB // 128) * qb:  # diagonal: clip+mask window
                    ctw = smp.tile([128, 128], F32, tag="ctw", bufs=2,
                                   name=f"ctw{h}_{qb}_{kc}")
                    nc.vector.tensor_scalar(ctw[:], et[:, off:off + 128],
                                            E_HI, E_LO, op0=OP.min, op1=OP.max)
                    nc.vector.tensor_tensor(pt[:, off:off + 128], ctw[:],
                                            mask_t[:], op=OP.mult)
                    if w > 128:
                        nc.vector.tensor_scalar(pt[:, off + 128:],
                                                et[:, off + 128:], E_HI, E_LO,
                                                op0=OP.min, op1=OP.max)
                else:
                    nc.vector.tensor_scalar(pt[:], et[:], E_HI, E_LO,
                                            op0=OP.min, op1=OP.max)
                if kc == 0:
                    nc.gpsimd.tensor_copy(sumacc[h][:], pt[:])
                else:
                    nc.gpsimd.tensor_tensor(sumacc[h][:, off:],
                                            sumacc[h][:, off:], pt[:, off:],
                                            op=OP.add)
                pend.append((h, kc, off, pt))
        while len(pend) > 2:
            flush(False)
        while pend:
            flush(True)
        # collapse sums: [128, QB] -> [1, QB] per head
        psums = []
        for h in range(HC):
            pt_s = ps_p.tile([128, QB], F32, tag="pp", name=f"psum{h}_{qb}")
            nc.tensor.matmul(pt_s[0:1, :], o128_t[:], sumacc[h][:],
                             start=True, stop=True)
            psums.append(pt_s)
        return pat, psums

    def tail_recip(qb, h, psum):
        """DVE part of the softmax tail: reciprocal of the denominators."""
        rc = smp.tile([1, QB], F32, tag=f"rc{h}", bufs=2, name=f"rc{h}_{qb}")
        nc.vector.reciprocal_approx_fast(rc[:], psum[0:1, :])
        rcr = smp.tile([1, QB], F32R, tag=f"rcr{h}", bufs=2,
                       name=f"rcr{h}_{qb}")
        nc.vector.tensor_copy(rcr[:], rc[:])
        return rcr

    def tail_norm(qb, h, pat, rcr, attn_n):
        """Broadcast 1/sum along partitions (tiny matmul) and normalize."""
        prb = ps_s.tile([128, QB], F32, tag="ss", name=f"prb{h}_{qb}")
        nc.tensor.matmul(prb[:], o1_t[:], rcr[:], start=True, stop=True)
        rbs = smp.tile([128, QB], F32, tag=f"rbs{h}", bufs=2,
                       name=f"rbs{h}_{qb}")
        nc.scalar.copy(rbs[:], prb[:])
        nc.vector.tensor_tensor(attn_n[:], pat[:], rbs[:], op=OP.mult)

    def outproj(qb, attn_n):
        qsl = slice(qb * QB, (qb + 1) * QB)
        for m in range(NMC):
            po = ps_p.tile([128, QB], F32, tag="pp", name=f"po{m}_{qb}")
            for h in range(HC):
                nc.tensor.matmul(po[:], wo_t[h][:, m * 128:(m + 1) * 128],
                                 attn_n[h][:], start=(h == 0),
                                 stop=(h == HC - 1))
            ob = o5p.tile([128, QB], F32, tag="ob", name=f"ob{m}_{qb}")
            # split evac across two engines so slot recycling outpaces PE
            nc.vector.tensor_copy(ob[:, 0:QB // 2], po[:, 0:QB // 2])
            nc.scalar.copy(ob[:, QB // 2:], po[:, QB // 2:])
            nc.sync.dma_start(d["outT"][m * 128:(m + 1) * 128, qsl], ob[:])

    # ---- software-pipelined main loop ----
    qct, qrt = proj(0)
    pending_o = None
    for qb in range(NQB):
        pat, psums = attn(qb, qct, qrt)
        attn_n = [prj.tile([128, QB], BF16, tag=f"an{h}", name=f"an{h}_{qb}")
                  for h in range(HC)]
        rcrs = [tail_recip(qb, h, psums[h]) for h in range(HC)]

        def mid(qb=qb, pat=pat, rcrs=rcrs, attn_n=attn_n):
            for h in range(HC):
                tail_norm(qb, h, pat[h], rcrs[h], attn_n[h][:])

        if qb < NQB - 1:
            qct, qrt = proj(qb + 1, mid_hook=mid)  # hides the softmax tail
            if pending_o is not None:
                outproj(*pending_o)
        else:
            if pending_o is not None:
                outproj(*pending_o)  # hides the last block's softmax tail
            mid()
        pending_o = (qb, attn_n)
    outproj(*pending_o)


def _prep_inputs(x, W_DQ, W_UQ, W_QR, W_DKV, W_UK, W_UV, W_KR, W_O):
    """Host-side weight fusion + sharding + mega-tile layout -> 8 in_maps."""
    f32 = np.float32
    bf16 = ml_dtypes.bfloat16
    perm = np.concatenate([np.arange(0, DHR, 2), np.arange(1, DHR, 2)])

    # x packed: [128, qb, m, col] so each (qb) slice and (qb,m) chunk is
    # contiguous along the free axis
    xT = x[0].astype(f32).T                             # [DM, S]
    xq = np.ascontiguousarray(
        xT.reshape(NMC, 128, NQB, QB).transpose(1, 2, 0, 3)
        .reshape(128, NQB * NMC * QB)).astype(bf16)

    # fused projection matrices (f32 on host)
    Aq = (W_UQ.astype(f32) @ W_DQ.astype(f32))      # [H*DH, DM]
    Ak = (W_UK.astype(f32) @ W_DKV.astype(f32))
    Av = (W_UV.astype(f32) @ W_DKV.astype(f32))
    Aqr = (W_QR.astype(f32) @ W_DQ.astype(f32))     # [H*DHR, DM]

    # rope tables (transposed, permuted-channel layout, 2x64 blocks)
    pos = np.arange(S, dtype=np.float64)
    inv = THETA ** (-np.arange(0, DHR, 2, dtype=np.float64) / DHR)  # (32,)
    ang = inv[:, None] * pos[None, :]                               # (32, S)
    cosv = np.cos(ang).astype(f32)
    sinv = np.sin(ang).astype(f32)
    blk1 = np.concatenate([cosv, cosv], axis=0)      # (64, S)
    blk2 = np.concatenate([-sinv, sinv], axis=0)
    cs1 = np.ascontiguousarray(np.concatenate([blk1, blk1], axis=0))
    cs2 = np.ascontiguousarray(np.concatenate([blk2, blk2], axis=0))

    kk = np.arange(128)[:, None]
    qq = np.arange(128)[None, :]
    masktri = np.ascontiguousarray((kk <= qq).astype(f32))

    wkrT = W_KR.astype(f32).T[:, perm]               # [DM, 64]

    def pack_chunks(a):
        """[DM, C] -> [128, NMC*C] with chunk m at cols m*C..(m+1)*C."""
        dmc = a.shape[1]
        return np.ascontiguousarray(
            a.reshape(NMC, 128, dmc).transpose(1, 0, 2)
            .reshape(128, NMC * dmc))

    shared = {
        "xq": xq, "masktri": masktri, "cs1": cs1, "cs2": cs2,
        "ones128": np.ones((128, 1), f32), "ones1": np.ones((1, 128), f32),
    }
    in_maps = []
    for c in range(N_CORES):
        hs = [c * HC + h for h in range(HC)]
        wq = np.concatenate(
            [Aq[h * DH:(h + 1) * DH, :].T for h in hs], axis=1)
        wk = np.concatenate(
            [Ak[h * DH:(h + 1) * DH, :].T for h in hs], axis=1)
        wv = np.concatenate(
            [Av[h * DH:(h + 1) * DH, :].T for h in hs], axis=1)
        wr = np.concatenate(
            [Aqr[h * DHR:(h + 1) * DHR, :].T[:, perm] for h in hs]
            + [wkrT], axis=1)                         # [DM, 192]
        woT = np.concatenate(
            [W_O[:, h * DH:(h + 1) * DH].astype(f32).T for h in hs], axis=0)
        in_maps.append({
            **shared,
            "wq": pack_chunks(wq).astype(bf16),
            "wk": pack_chunks(wk).astype(bf16),
            "wv": pack_chunks(wv).astype(bf16),
            "wr": pack_chunks(wr).astype(bf16),
            "woT": np.ascontiguousarray(woT).astype(bf16),
        })
    return in_maps


def kernel(**inputs):
    global LAST_EXEC_TIME_NS, LAST_RESULTS
    if "nc" not in _CACHE:
        _CACHE["nc"] = _build()
    nc = _CACHE["nc"]
    in_maps = _prep_inputs(**{k: np.asarray(v) for k, v in inputs.items()})
    kwargs = dict(TRACE_KWARGS)
    if TRACE:
        kwargs["trace"] = True
    res = run_bass_kernel_spmd(nc, in_maps, core_ids=list(range(N_CORES)),
                               **kwargs)
    LAST_EXEC_TIME_NS = res.exec_time_ns
    LAST_RESULTS = res
    acc = np.zeros((DM, S), np.float64)
    for c in range(N_CORES):
        acc += res.results[c]["outT"].astype(np.float64)
    return np.ascontiguousarray(acc.T[None]).astype(np.float32)
